# revision 35
# baseline (speedup 1.0000x reference)
"""Trainium2 Bass kernel for banded (local-causal) multi-head self-attention.

Problem (hardcoded shapes): x [4, 2048, 1024], W_attn [1024, 3072],
b_attn [3072], W_proj [1024, 1024], b_proj [1024]; 16 heads, head dim 64,
local causal window 256.

Sharding over 8 NeuronCores: data-parallel over the 4 batches x
tensor-parallel over 2 head-groups (8 heads each). Each core computes a
partial projection output [2048, 1024] (bf16); the host sums the two
head-group partials per batch and adds b_proj.

Per-core device program, one fused pipeline (all loops unrolled under Tile):
  prefix : v projections for token blocks 0-2 and the q/k projection tiles
           covering tokens 0-511, behind a handful of large packed DMAs.
  iter i : (i = key block 0..15)
    - S^T strips for all 4 head pairs: [64x128]^T @ [64, w] matmuls (the two
      heads of a pair sit in disjoint PE row-halves and run concurrently),
      Exp activation (scale 1/8) on scalar, band-mask multiply on vector.
    - PV for query block j=i: each pair accumulates [128, 2*(64+1)] in PSUM
      from the last three strips (ones column per head emits the softmax
      denominator), then reciprocal + per-head normalize into a_nat, and a
      DMA transpose into the feature-major aTb tiles.
    - interleaved dense work to keep the PE warm: v projection for block
      i+3, the remaining q/k tiles (2 per iter), and the output projection
      of block j=i-1 (from aTb, bf16 result tiles DMA'd out via gpsimd).
  tail   : output projection of block 15.
"""

import numpy as np
import ml_dtypes

import concourse.bass as bass
import concourse.bacc as bacc
import concourse.mybir as mybir
import concourse.tile as tile
from concourse.bass_utils import run_bass_kernel_spmd

B, T, C = 4, 2048, 1024
H, D, CTX = 16, 64, 256
HG = 8                 # heads per core
FG = HG * D            # 512 features per group
P = 128
NT = T // P            # 16 token blocks
KC = C // P            # 8 contraction tiles of C
W3 = 3 * P             # strip width 384

BF16 = mybir.dt.bfloat16
F32 = mybir.dt.float32

# set by the last kernel() call; test harness reads exec_time_ns from here
LAST_RESULTS = None

_BUILD_CACHE = {}


def _build_nc(qk_bias: bool, v_bias: bool) -> bass.Bass:
    nc = bacc.Bacc()

    # xt layout: [P, tier(4) * k(8) * 512] — tier-major so each 512-token
    # tier is one contiguous DMA into its own tile
    xt_d = nc.declare_dram_parameter("xt", [P, KC * T], BF16, isOutput=False)
    # wqk split: q features then k features, each [P, k(8) * 512]
    wqkq_d = nc.declare_dram_parameter("wqkq", [P, KC * FG], BF16, isOutput=False)
    wqkk_d = nc.declare_dram_parameter("wqkk", [P, KC * FG], BF16, isOutput=False)
    wv_d = nc.declare_dram_parameter("wv", [P, KC * FG], BF16, isOutput=False)
    wp_d = nc.declare_dram_parameter("wp", [P, 4 * C], BF16, isOutput=False)
    # [lower-tri | strict-upper-tri] pair of 128x128 band-mask blocks
    mstrip_d = nc.declare_dram_parameter("mstrip", [P, 2 * P], BF16, isOutput=False)
    ident_d = nc.declare_dram_parameter("ident", [P, P], BF16, isOutput=False)
    if qk_bias:
        bqk_d = nc.declare_dram_parameter("bqk", [8, P], F32, isOutput=False)
    if v_bias:
        bv_d = nc.declare_dram_parameter("bv", [P, HG * (D + 1)], BF16, isOutput=False)
    y_d = nc.declare_dram_parameter("y", [T, C], BF16, isOutput=True)

    with tile.TileContext(nc) as tc:
        with tc.tile_pool(name="const", bufs=1) as const, \
             tc.tile_pool(name="stage", bufs=6) as stage_p, \
             tc.tile_pool(name="anpool", bufs=3) as an_pool, \
             tc.tile_pool(name="ypool", bufs=3) as y_pool:

            # ---- resident SBUF tiles -------------------------------------
            xtt = [const.tile([P, KC * 512], BF16, tag=f"xt{r}", name=f"xt{r}")
                   for r in range(4)]
            wqkq = const.tile([P, KC * FG], BF16, tag="wqkq", name="wqkq")
            wqkk = const.tile([P, KC * FG], BF16, tag="wqkk", name="wqkk")
            wvb = const.tile([P, KC * FG], BF16, tag="wv", name="wv")
            wpb = const.tile([P, 4 * C], BF16, tag="wp", name="wp")
            qkT = [const.tile([P, T], BF16, tag=f"qkT{f}", name=f"qkT{f}") for f in range(8)]
            vag = [const.tile([P, HG * (D + 1)], BF16, tag=f"vag{t}", name=f"vag{t}") for t in range(NT)]
            aTbig = const.tile([P, 4 * T], BF16, tag="aTb", name="aTb")
            aTb = [aTbig[:, k * T:(k + 1) * T] for k in range(FG // P)]
            mask_t = const.tile([P, 2 * P], BF16, tag="mask", name="mask")
            ident_t = const.tile([P, P], BF16, tag="ident", name="ident")
            # rolling E strips, one window of 3 per (pair, head-in-pair)
            estr = {(hp, idx, s): const.tile([P, W3], BF16, tag=f"e{hp}_{idx}_{s}",
                                             name=f"e{hp}_{idx}_{s}")
                    for hp in range(4) for idx in range(2) for s in range(3)}

            def xt_(k, c0, c1):
                # token columns [c0, c1) of contraction chunk k; must not
                # cross a 512-token tier boundary
                r = c0 // 512
                assert c1 <= (r + 1) * 512
                return xtt[r][:, k * 512 + c0 - r * 512:k * 512 + c1 - r * 512]

            def wqk_(ft, k):
                wb = wqkq if ft < 4 else wqkk
                f = ft % 4
                return wb[:, k * FG + f * P:k * FG + (f + 1) * P]

            def wv_(k):
                return wvb[:, k * FG:(k + 1) * FG]

            def wp_(k2):
                return wpb[:, k2 * C:(k2 + 1) * C]

            # ---- input DMAs: a few large packed transfers ----------------
            # ordered so the startup-critical pieces (wv + first token tier +
            # k-side then q-side qk weights) land first
            nc.sync.dma_start(mask_t[:], mstrip_d[:])
            nc.sync.dma_start(ident_t[:], ident_d[:])
            nc.sync.dma_start(wvb[:], wv_d[:])
            nc.sync.dma_start(xtt[0][:], xt_d[:, 0:KC * 512])
            nc.sync.dma_start(xtt[1][:], xt_d[:, KC * 512:2 * KC * 512])
            nc.sync.dma_start(xtt[2][:], xt_d[:, 2 * KC * 512:3 * KC * 512])
            nc.scalar.dma_start(wqkq[:], wqkq_d[:])
            nc.scalar.dma_start(wqkk[:], wqkk_d[:])
            nc.scalar.dma_start(wpb[:], wp_d[:])
            nc.scalar.dma_start(xtt[3][:], xt_d[:, 3 * KC * 512:4 * KC * 512])
            if qk_bias:
                bqk_t = const.tile([P, 8], F32, tag="bqk", name="bqk")
                nc.scalar.dma_start(bqk_t[:], bqk_d.rearrange("a p -> p a"))
            if v_bias:
                bv_t = const.tile([P, HG * (D + 1)], BF16, tag="bv", name="bv")
                nc.scalar.dma_start(bv_t[:], bv_d[:])

            with tc.tile_pool(name="ps_big", bufs=2, space="PSUM") as ps_big, \
                 tc.tile_pool(name="ps_s", bufs=3, space="PSUM") as ps_s, \
                 tc.tile_pool(name="ps_a", bufs=2, space="PSUM") as ps_a, \
                 tc.tile_pool(name="ps_tr", bufs=1, space="PSUM") as ps_tr:

                def emit_v(t):
                    ps = ps_big.tile([P, FG], F32, tag="big", name="psv")
                    for k in range(KC):
                        nc.tensor.matmul(
                            ps[:],
                            lhsT=xt_(k, t * P, (t + 1) * P),
                            rhs=wv_(k),
                            start=(k == 0),
                            stop=(k == KC - 1),
                        )
                    vv = vag[t].rearrange("p (h c) -> p h c", c=D + 1)
                    nc.gpsimd.memset(vv[:, :, D:D + 1], 1.0)
                    nc.vector.tensor_copy(
                        vv[:, :, 0:D],
                        ps.rearrange("p (h c) -> p h c", c=D),
                    )
                    if v_bias:
                        nc.vector.tensor_add(vag[t][:], vag[t][:], bv_t[:])

                def emit_qk(ft, nt, on_act):
                    ps = ps_big.tile([P, FG], F32, tag="big", name="psqk")
                    for k in range(KC):
                        nc.tensor.matmul(
                            ps[:],
                            lhsT=wqk_(ft, k),
                            rhs=xt_(k, nt * 512, (nt + 1) * 512),
                            start=(k == 0),
                            stop=(k == KC - 1),
                        )
                    dst = qkT[ft][:, nt * 512:(nt + 1) * 512]
                    if qk_bias:
                        nc.scalar.activation(
                            dst, ps[:],
                            mybir.ActivationFunctionType.Copy,
                            bias=bqk_t[:, ft:ft + 1],
                        )
                    elif on_act:
                        nc.scalar.activation(
                            dst, ps[:], mybir.ActivationFunctionType.Copy)
                    else:
                        nc.vector.tensor_copy(dst, ps[:])

                def emit_s(hp, i):
                    w = min(W3, (NT - i) * P)
                    for idx in range(2):
                        ho = idx * D
                        ps = ps_s.tile([P, W3], F32, tag="s", name="pss")
                        nc.tensor.matmul(
                            ps[:, :w],
                            lhsT=qkT[4 + hp][ho:ho + D, i * P:(i + 1) * P],
                            rhs=qkT[hp][ho:ho + D, i * P:i * P + w],
                            start=True, stop=True,
                        )
                        e_t = estr[(hp, idx, i % 3)]
                        nc.scalar.activation(
                            e_t[:, :w], ps[:, :w],
                            mybir.ActivationFunctionType.Exp,
                            scale=0.125,
                        )
                        # band mask: cols 0:128 lower-tri, 256:384 strict
                        # upper-tri, middle 128 all-valid (left untouched);
                        # idx 0 on gpsimd (otherwise idle), idx 1 on vector
                        eng = nc.gpsimd if idx == 0 else nc.vector
                        if w == W3:
                            e3 = e_t.rearrange("p (a b) -> p a b", b=P)[:, ::2, :]
                            m3 = mask_t.rearrange("p (a b) -> p a b", b=P)
                            eng.tensor_mul(e3, e3, m3)
                        else:
                            eng.tensor_mul(e_t[:, 0:P], e_t[:, 0:P],
                                           mask_t[:, 0:P])

                def emit_pv(hp, i):
                    # query block j = i accumulated from strips i-2, i-1, i
                    psA = ps_a.tile([P, 2 * (D + 1)], F32, tag="a", name="psA")
                    ds = [d for d in (2, 1, 0) if i - d >= 0]
                    for n, d in enumerate(ds):
                        for idx in range(2):
                            h = 2 * hp + idx
                            nc.tensor.matmul(
                                psA[:, idx * (D + 1):(idx + 1) * (D + 1)],
                                lhsT=estr[(hp, idx, (i - d) % 3)][:, d * P:(d + 1) * P],
                                rhs=vag[i - d][:, h * (D + 1):(h + 1) * (D + 1)],
                                start=(n == 0 and idx == 0),
                                stop=(n == len(ds) - 1 and idx == 1),
                                skip_group_check=True,
                            )
                    return psA

                def emit_drain(hp, i, psA, an):
                    pa3 = psA.rearrange("p (i c) -> p i c", c=D + 1)
                    rs = stage_p.tile([P, 2], F32, tag="rs", name="rs")
                    nc.vector.reciprocal(rs[:], pa3[:, :, D])
                    for idx in range(2):
                        h = 2 * hp + idx
                        nc.vector.tensor_scalar(
                            an[:, h * D:(h + 1) * D],
                            psA[:, idx * (D + 1):idx * (D + 1) + D],
                            rs[:, idx:idx + 1],
                            None,
                            mybir.AluOpType.mult,
                        )

                def emit_transpose(i, an, hp, pst):
                    # PE transpose into a quarter of the shared bf16 psum
                    # bank; only hp 0 may clear the bank
                    nc.tensor.matmul(
                        pst[:, hp * P:(hp + 1) * P],
                        lhsT=an[:, hp * P:(hp + 1) * P],
                        rhs=ident_t[:],
                        is_transpose=True,
                        start=(hp == 0),
                        stop=(hp == 3),
                        skip_group_check=True,
                    )
                    dst = aTb[hp][:, i * P:(i + 1) * P]
                    src = pst[:, hp * P:(hp + 1) * P]
                    if hp % 2 == 0:
                        nc.vector.tensor_copy(dst, src)
                    else:
                        nc.scalar.activation(
                            dst, src, mybir.ActivationFunctionType.Copy)

                def emit_proj(j, n, yt):
                    ps2 = ps_big.tile([P, FG], F32, tag="big", name="psp")
                    for k2 in range(FG // P):
                        nc.tensor.matmul(
                            ps2[:],
                            lhsT=aTb[k2][:, j * P:(j + 1) * P],
                            rhs=wp_(k2)[:, n * 512:(n + 1) * 512],
                            start=(k2 == 0),
                            stop=(k2 == FG // P - 1),
                        )
                    dst = yt[:, n * 512:(n + 1) * 512]
                    if n == 0:
                        nc.scalar.activation(
                            dst, ps2[:], mybir.ActivationFunctionType.Copy)
                    else:
                        nc.vector.tensor_copy(dst, ps2[:])
                        nc.gpsimd.dma_start(y_d[j * P:(j + 1) * P, :], yt[:])

                # ---- prefix --------------------------------------------
                # q-side weight tiles land first (scalar queue head), k-side
                # follows; ordering the prefix to match the DMA arrival keeps
                # the in-order PE stream from blocking on late transfers
                for t in range(3):
                    emit_v(t)
                for ft in (0, 1, 2, 3, 4, 5, 6, 7):
                    emit_qk(ft, 0, on_act=True)

                # qk tiles still to emit, with the iteration each is
                # sprinkled into (deadlines: nt1 by iter 2, nt2 by 6, nt3 by
                # 10); spread thin so late iters keep dense PE filler
                qk_sched = {}
                order = [(ft, nt) for nt in (1, 2, 3)
                         for ft in (4, 0, 5, 1, 6, 2, 7, 3)]
                slots = ([0, 0, 0, 0, 1, 1, 1, 1]     # nt1 iters 0-1
                         + [2, 2, 3, 3, 4, 4, 5, 5]   # nt2 iters 2-5
                         + [6, 6, 7, 7, 8, 8, 9, 9])  # nt3 iters 6-9
                for (ft, nt), it in zip(order, slots):
                    qk_sched.setdefault(it, []).append((ft, nt))

                # ---- fused attention + projection loop ------------------
                for i in range(NT):
                    emit_s(0, i)
                    emit_s(1, i)
                    if 2 <= i <= 14:
                        emit_v(i + 1)
                    emit_s(2, i)
                    emit_s(3, i)

                    an = an_pool.tile([P, FG], BF16, tag="an", name="an")
                    pst = ps_tr.tile([P, FG], BF16, tag="tr", name="pst")
                    sprinkle = qk_sched.get(i, [])

                    psA = emit_pv(0, i)
                    emit_drain(0, i, psA, an)
                    emit_transpose(i, an, 0, pst)
                    psA = emit_pv(1, i)
                    emit_drain(1, i, psA, an)
                    emit_transpose(i, an, 1, pst)
                    for ft, nt in sprinkle[:len(sprinkle) // 2]:
                        emit_qk(ft, nt, on_act=(ft % 2 == 0))
                    psA = emit_pv(2, i)
                    emit_drain(2, i, psA, an)
                    emit_transpose(i, an, 2, pst)
                    psA = emit_pv(3, i)
                    emit_drain(3, i, psA, an)
                    emit_transpose(i, an, 3, pst)
                    for ft, nt in sprinkle[len(sprinkle) // 2:]:
                        emit_qk(ft, nt, on_act=(ft % 2 == 0))
                    if i >= 1:
                        yt = y_pool.tile([P, C], BF16, tag="y", name="y")
                        emit_proj(i - 1, 0, yt)
                        emit_proj(i - 1, 1, yt)

                yt = y_pool.tile([P, C], BF16, tag="y", name="y")
                emit_proj(NT - 1, 0, yt)
                emit_proj(NT - 1, 1, yt)

    nc.finalize()
    return nc


def _band_mask_strip() -> np.ndarray:
    tk = np.arange(P)[:, None]
    tq = np.arange(P)[None, :]
    tri = (tq >= tk)            # dj=0 block: lower-incl triangle valid
    striu = (tq < tk)           # dj=2 block: strict upper triangle valid
    return np.concatenate([tri, striu], axis=1).astype(ml_dtypes.bfloat16)


def _pack_rows(a: np.ndarray) -> np.ndarray:
    """[n*P, F] -> [P, n*F] with chunk-major free dim (chunk k at cols k*F)."""
    n = a.shape[0] // P
    return np.ascontiguousarray(
        a.reshape(n, P, a.shape[1]).transpose(1, 0, 2).reshape(P, -1)
    )


def kernel(x, W_attn, b_attn, W_proj, b_proj):
    global LAST_RESULTS
    x = np.asarray(x, dtype=np.float32)
    W_attn = np.asarray(W_attn, dtype=np.float32)
    b_attn = np.asarray(b_attn, dtype=np.float32)
    W_proj = np.asarray(W_proj, dtype=np.float32)
    b_proj = np.asarray(b_proj, dtype=np.float32)

    qk_bias = bool(np.any(b_attn[:2 * C]))
    v_bias = bool(np.any(b_attn[2 * C:]))

    key = (qk_bias, v_bias)
    if key not in _BUILD_CACHE:
        _BUILD_CACHE[key] = _build_nc(qk_bias, v_bias)
    nc = _BUILD_CACHE[key]

    mstrip = _band_mask_strip()
    in_maps = []
    for c in range(8):
        b, g = c // 2, c % 2
        fsl = slice(FG * g, FG * (g + 1))
        # xt: [C, T] -> [P, tier(4) k(8) t(512)]
        xtp = np.ascontiguousarray(
            x[b].T.reshape(KC, P, 4, 512).transpose(1, 2, 0, 3).reshape(P, KC * T)
        )
        im = {
            "xt": xtp.astype(ml_dtypes.bfloat16),
            "wqkq": _pack_rows(W_attn[:, fsl]).astype(ml_dtypes.bfloat16),
            "wqkk": _pack_rows(
                W_attn[:, C + FG * g:C + FG * (g + 1)]
            ).astype(ml_dtypes.bfloat16),
            "wv": _pack_rows(
                W_attn[:, 2 * C + FG * g:2 * C + FG * (g + 1)]
            ).astype(ml_dtypes.bfloat16),
            "wp": _pack_rows(W_proj[fsl, :]).astype(ml_dtypes.bfloat16),
            "mstrip": mstrip,
            "ident": np.eye(P, dtype=ml_dtypes.bfloat16),
        }
        if qk_bias:
            bq = b_attn[fsl]
            bk = b_attn[C + FG * g:C + FG * (g + 1)]
            im["bqk"] = np.concatenate([bq, bk]).reshape(8, P).astype(np.float32)
        if v_bias:
            bv = b_attn[2 * C + FG * g:2 * C + FG * (g + 1)]
            bvt = np.zeros((HG, D + 1), dtype=np.float32)
            bvt[:, 1:] = bv.reshape(HG, D)
            im["bv"] = np.broadcast_to(
                bvt.reshape(1, HG * (D + 1)), (P, HG * (D + 1))
            ).astype(ml_dtypes.bfloat16)
        in_maps.append(im)

    res = run_bass_kernel_spmd(nc, in_maps, list(range(8)))
    LAST_RESULTS = res

    out = np.empty((B, T, C), dtype=np.float32)
    for b in range(B):
        out[b] = (res.results[2 * b]["y"].astype(np.float32)
                  + res.results[2 * b + 1]["y"].astype(np.float32) + b_proj)
    return out


# revision 36
# speedup vs baseline: 1.0331x; 1.0331x over previous
"""Trainium2 Bass kernel for banded (local-causal) multi-head self-attention.

Problem (hardcoded shapes): x [4, 2048, 1024], W_attn [1024, 3072],
b_attn [3072], W_proj [1024, 1024], b_proj [1024]; 16 heads, head dim 64,
local causal window 256.

Sharding over 8 NeuronCores: data-parallel over the 4 batches x
tensor-parallel over 2 head-groups (8 heads each). Each core computes a
partial projection output [2048, 1024] (bf16); the host sums the two
head-group partials per batch and adds b_proj.

Per-core device program, one fused pipeline (all loops unrolled under Tile):
  prefix : v projections for token blocks 0-2 and the q/k projection tiles
           covering tokens 0-511, behind a handful of large packed DMAs.
  iter i : (i = key block 0..15)
    - S^T strips for all 4 head pairs: [64x128]^T @ [64, w] matmuls (the two
      heads of a pair sit in disjoint PE row-halves and run concurrently),
      Exp activation (scale 1/8) on scalar, band-mask multiply on vector.
    - PV for query block j=i: each pair accumulates [128, 2*(64+1)] in PSUM
      from the last three strips (ones column per head emits the softmax
      denominator), then reciprocal + per-head normalize into a_nat, and a
      DMA transpose into the feature-major aTb tiles.
    - interleaved dense work to keep the PE warm: v projection for block
      i+3, the remaining q/k tiles (2 per iter), and the output projection
      of block j=i-1 (from aTb, bf16 result tiles DMA'd out via gpsimd).
  tail   : output projection of block 15.
"""

import numpy as np
import ml_dtypes

import concourse.bass as bass
import concourse.bacc as bacc
import concourse.mybir as mybir
import concourse.tile as tile
from concourse.bass_utils import run_bass_kernel_spmd

B, T, C = 4, 2048, 1024
H, D, CTX = 16, 64, 256
HG = 8                 # heads per core
FG = HG * D            # 512 features per group
P = 128
NT = T // P            # 16 token blocks
KC = C // P            # 8 contraction tiles of C
W3 = 3 * P             # strip width 384

BF16 = mybir.dt.bfloat16
F32 = mybir.dt.float32

# set by the last kernel() call; test harness reads exec_time_ns from here
LAST_RESULTS = None

_BUILD_CACHE = {}


def _build_nc(qk_bias: bool, v_bias: bool) -> bass.Bass:
    nc = bacc.Bacc()

    # xt layout: [P, tier(4) * k(8) * 512] — tier-major so each 512-token
    # tier is one contiguous DMA into its own tile
    xt_d = nc.declare_dram_parameter("xt", [P, KC * T], BF16, isOutput=False)
    # wqk split: q features then k features, each [P, k(8) * 512]
    wqkq_d = nc.declare_dram_parameter("wqkq", [P, KC * FG], BF16, isOutput=False)
    wqkk_d = nc.declare_dram_parameter("wqkk", [P, KC * FG], BF16, isOutput=False)
    wv_d = nc.declare_dram_parameter("wv", [P, KC * FG], BF16, isOutput=False)
    wp_d = nc.declare_dram_parameter("wp", [P, 4 * C], BF16, isOutput=False)
    # [lower-tri | strict-upper-tri] pair of 128x128 band-mask blocks
    mstrip_d = nc.declare_dram_parameter("mstrip", [P, 2 * P], BF16, isOutput=False)
    ident_d = nc.declare_dram_parameter("ident", [P, P], BF16, isOutput=False)
    if qk_bias:
        bqk_d = nc.declare_dram_parameter("bqk", [8, P], F32, isOutput=False)
    if v_bias:
        bv_d = nc.declare_dram_parameter("bv", [P, HG * (D + 1)], BF16, isOutput=False)
    y_d = nc.declare_dram_parameter("y", [T, C], BF16, isOutput=True)

    with tile.TileContext(nc) as tc:
        with tc.tile_pool(name="const", bufs=1) as const, \
             tc.tile_pool(name="stage", bufs=6) as stage_p, \
             tc.tile_pool(name="anpool", bufs=3) as an_pool, \
             tc.tile_pool(name="ypool", bufs=3) as y_pool:

            # ---- resident SBUF tiles -------------------------------------
            xtt = [const.tile([P, KC * 512], BF16, tag=f"xt{r}", name=f"xt{r}")
                   for r in range(4)]
            wqkq = const.tile([P, KC * FG], BF16, tag="wqkq", name="wqkq")
            wqkk = const.tile([P, KC * FG], BF16, tag="wqkk", name="wqkk")
            wvb = const.tile([P, KC * FG], BF16, tag="wv", name="wv")
            wpb = const.tile([P, 4 * C], BF16, tag="wp", name="wp")
            qkT = [const.tile([P, T], BF16, tag=f"qkT{f}", name=f"qkT{f}") for f in range(8)]
            vag = [const.tile([P, HG * (D + 1)], BF16, tag=f"vag{t}", name=f"vag{t}") for t in range(NT)]
            aTbig = const.tile([P, 4 * T], BF16, tag="aTb", name="aTb")
            aTb = [aTbig[:, k * T:(k + 1) * T] for k in range(FG // P)]
            mask_t = const.tile([P, 2 * P], BF16, tag="mask", name="mask")
            ident_t = const.tile([P, P], BF16, tag="ident", name="ident")
            # rolling E strips, one window of 3 per (pair, head-in-pair)
            estr = {(hp, idx, s): const.tile([P, W3], BF16, tag=f"e{hp}_{idx}_{s}",
                                             name=f"e{hp}_{idx}_{s}")
                    for hp in range(4) for idx in range(2) for s in range(3)}

            def xt_(k, c0, c1):
                # token columns [c0, c1) of contraction chunk k; must not
                # cross a 512-token tier boundary
                r = c0 // 512
                assert c1 <= (r + 1) * 512
                return xtt[r][:, k * 512 + c0 - r * 512:k * 512 + c1 - r * 512]

            def wqk_(ft, k):
                wb = wqkq if ft < 4 else wqkk
                f = ft % 4
                return wb[:, k * FG + f * P:k * FG + (f + 1) * P]

            def wv_(k):
                return wvb[:, k * FG:(k + 1) * FG]

            def wp_(k2):
                return wpb[:, k2 * C:(k2 + 1) * C]

            # ---- input DMAs: a few large packed transfers ----------------
            # ordered so the startup-critical pieces (wv + first token tier +
            # k-side then q-side qk weights) land first
            # weights stream on the sync queue while x tiers stream on the
            # scalar queue, so the first v matmuls (wv + tier0) unblock as
            # early as the ~8us DMA launch latency allows
            nc.sync.dma_start(mask_t[:], mstrip_d[:])
            nc.sync.dma_start(ident_t[:], ident_d[:])
            nc.sync.dma_start(wvb[:], wv_d[:])
            nc.sync.dma_start(wqkq[:], wqkq_d[:])
            nc.sync.dma_start(wqkk[:], wqkk_d[:])
            nc.sync.dma_start(xtt[2][:], xt_d[:, 2 * KC * 512:3 * KC * 512])
            nc.scalar.dma_start(xtt[0][:], xt_d[:, 0:KC * 512])
            nc.scalar.dma_start(xtt[1][:], xt_d[:, KC * 512:2 * KC * 512])
            nc.scalar.dma_start(wpb[:], wp_d[:])
            nc.scalar.dma_start(xtt[3][:], xt_d[:, 3 * KC * 512:4 * KC * 512])
            if qk_bias:
                bqk_t = const.tile([P, 8], F32, tag="bqk", name="bqk")
                nc.scalar.dma_start(bqk_t[:], bqk_d.rearrange("a p -> p a"))
            if v_bias:
                bv_t = const.tile([P, HG * (D + 1)], BF16, tag="bv", name="bv")
                nc.scalar.dma_start(bv_t[:], bv_d[:])

            with tc.tile_pool(name="ps_big", bufs=2, space="PSUM") as ps_big, \
                 tc.tile_pool(name="ps_s", bufs=3, space="PSUM") as ps_s, \
                 tc.tile_pool(name="ps_a", bufs=2, space="PSUM") as ps_a, \
                 tc.tile_pool(name="ps_tr", bufs=1, space="PSUM") as ps_tr:

                def emit_v(t):
                    ps = ps_big.tile([P, FG], F32, tag="big", name="psv")
                    for k in range(KC):
                        nc.tensor.matmul(
                            ps[:],
                            lhsT=xt_(k, t * P, (t + 1) * P),
                            rhs=wv_(k),
                            start=(k == 0),
                            stop=(k == KC - 1),
                        )
                    vv = vag[t].rearrange("p (h c) -> p h c", c=D + 1)
                    nc.gpsimd.memset(vv[:, :, D:D + 1], 1.0)
                    nc.vector.tensor_copy(
                        vv[:, :, 0:D],
                        ps.rearrange("p (h c) -> p h c", c=D),
                    )
                    if v_bias:
                        nc.vector.tensor_add(vag[t][:], vag[t][:], bv_t[:])

                def emit_qk(ft, nt, on_act):
                    ps = ps_big.tile([P, FG], F32, tag="big", name="psqk")
                    for k in range(KC):
                        nc.tensor.matmul(
                            ps[:],
                            lhsT=wqk_(ft, k),
                            rhs=xt_(k, nt * 512, (nt + 1) * 512),
                            start=(k == 0),
                            stop=(k == KC - 1),
                        )
                    dst = qkT[ft][:, nt * 512:(nt + 1) * 512]
                    if qk_bias:
                        nc.scalar.activation(
                            dst, ps[:],
                            mybir.ActivationFunctionType.Copy,
                            bias=bqk_t[:, ft:ft + 1],
                        )
                    elif on_act:
                        nc.scalar.activation(
                            dst, ps[:], mybir.ActivationFunctionType.Copy)
                    else:
                        nc.vector.tensor_copy(dst, ps[:])

                def emit_s(hp, i):
                    w = min(W3, (NT - i) * P)
                    for idx in range(2):
                        ho = idx * D
                        ps = ps_s.tile([P, W3], F32, tag="s", name="pss")
                        nc.tensor.matmul(
                            ps[:, :w],
                            lhsT=qkT[4 + hp][ho:ho + D, i * P:(i + 1) * P],
                            rhs=qkT[hp][ho:ho + D, i * P:i * P + w],
                            start=True, stop=True,
                        )
                        e_t = estr[(hp, idx, i % 3)]
                        nc.scalar.activation(
                            e_t[:, :w], ps[:, :w],
                            mybir.ActivationFunctionType.Exp,
                            scale=0.125,
                        )
                        # band mask: cols 0:128 lower-tri, 256:384 strict
                        # upper-tri, middle 128 all-valid (left untouched);
                        # idx 0 on gpsimd (otherwise idle), idx 1 on vector
                        eng = nc.gpsimd if idx == 0 else nc.vector
                        if w == W3:
                            e3 = e_t.rearrange("p (a b) -> p a b", b=P)[:, ::2, :]
                            m3 = mask_t.rearrange("p (a b) -> p a b", b=P)
                            eng.tensor_mul(e3, e3, m3)
                        else:
                            eng.tensor_mul(e_t[:, 0:P], e_t[:, 0:P],
                                           mask_t[:, 0:P])

                def emit_pv(hp, i):
                    # query block j = i accumulated from strips i-2, i-1, i
                    psA = ps_a.tile([P, 2 * (D + 1)], F32, tag="a", name="psA")
                    ds = [d for d in (2, 1, 0) if i - d >= 0]
                    for n, d in enumerate(ds):
                        for idx in range(2):
                            h = 2 * hp + idx
                            nc.tensor.matmul(
                                psA[:, idx * (D + 1):(idx + 1) * (D + 1)],
                                lhsT=estr[(hp, idx, (i - d) % 3)][:, d * P:(d + 1) * P],
                                rhs=vag[i - d][:, h * (D + 1):(h + 1) * (D + 1)],
                                start=(n == 0 and idx == 0),
                                stop=(n == len(ds) - 1 and idx == 1),
                                skip_group_check=True,
                            )
                    return psA

                def emit_drain(hp, i, psA, an):
                    pa3 = psA.rearrange("p (i c) -> p i c", c=D + 1)
                    rs = stage_p.tile([P, 2], F32, tag="rs", name="rs")
                    nc.vector.reciprocal(rs[:], pa3[:, :, D])
                    for idx in range(2):
                        h = 2 * hp + idx
                        nc.vector.tensor_scalar(
                            an[:, h * D:(h + 1) * D],
                            psA[:, idx * (D + 1):idx * (D + 1) + D],
                            rs[:, idx:idx + 1],
                            None,
                            mybir.AluOpType.mult,
                        )

                def emit_transpose(i, an, hp, pst):
                    # PE transpose into a quarter of the shared bf16 psum
                    # bank; only hp 0 may clear the bank
                    nc.tensor.matmul(
                        pst[:, hp * P:(hp + 1) * P],
                        lhsT=an[:, hp * P:(hp + 1) * P],
                        rhs=ident_t[:],
                        is_transpose=True,
                        start=(hp == 0),
                        stop=(hp == 3),
                        skip_group_check=True,
                    )
                    dst = aTb[hp][:, i * P:(i + 1) * P]
                    src = pst[:, hp * P:(hp + 1) * P]
                    if hp % 2 == 0:
                        nc.vector.tensor_copy(dst, src)
                    else:
                        nc.scalar.activation(
                            dst, src, mybir.ActivationFunctionType.Copy)

                def emit_proj(j, n, yt):
                    ps2 = ps_big.tile([P, FG], F32, tag="big", name="psp")
                    for k2 in range(FG // P):
                        nc.tensor.matmul(
                            ps2[:],
                            lhsT=aTb[k2][:, j * P:(j + 1) * P],
                            rhs=wp_(k2)[:, n * 512:(n + 1) * 512],
                            start=(k2 == 0),
                            stop=(k2 == FG // P - 1),
                        )
                    dst = yt[:, n * 512:(n + 1) * 512]
                    if n == 0:
                        nc.scalar.activation(
                            dst, ps2[:], mybir.ActivationFunctionType.Copy)
                    else:
                        nc.vector.tensor_copy(dst, ps2[:])
                        nc.gpsimd.dma_start(y_d[j * P:(j + 1) * P, :], yt[:])

                # ---- prefix --------------------------------------------
                # q-side weight tiles land first (scalar queue head), k-side
                # follows; ordering the prefix to match the DMA arrival keeps
                # the in-order PE stream from blocking on late transfers
                for t in range(3):
                    emit_v(t)
                for ft in (0, 1, 2, 3, 4, 5, 6, 7):
                    emit_qk(ft, 0, on_act=True)

                # qk tiles still to emit, with the iteration each is
                # sprinkled into (deadlines: nt1 by iter 2, nt2 by 6, nt3 by
                # 10); spread thin so late iters keep dense PE filler
                qk_sched = {}
                order = [(ft, nt) for nt in (1, 2, 3)
                         for ft in (4, 0, 5, 1, 6, 2, 7, 3)]
                slots = ([0, 0, 0, 0, 1, 1, 1, 1]     # nt1 iters 0-1
                         + [2, 2, 3, 3, 4, 4, 5, 5]   # nt2 iters 2-5
                         + [6, 6, 7, 7, 8, 8, 9, 9])  # nt3 iters 6-9
                for (ft, nt), it in zip(order, slots):
                    qk_sched.setdefault(it, []).append((ft, nt))

                # ---- fused attention + projection loop ------------------
                for i in range(NT):
                    emit_s(0, i)
                    emit_s(1, i)
                    if 2 <= i <= 14:
                        emit_v(i + 1)
                    emit_s(2, i)
                    emit_s(3, i)

                    an = an_pool.tile([P, FG], BF16, tag="an", name="an")
                    pst = ps_tr.tile([P, FG], BF16, tag="tr", name="pst")
                    sprinkle = qk_sched.get(i, [])

                    psA = emit_pv(0, i)
                    emit_drain(0, i, psA, an)
                    emit_transpose(i, an, 0, pst)
                    psA = emit_pv(1, i)
                    emit_drain(1, i, psA, an)
                    emit_transpose(i, an, 1, pst)
                    for ft, nt in sprinkle[:len(sprinkle) // 2]:
                        emit_qk(ft, nt, on_act=(ft % 2 == 0))
                    psA = emit_pv(2, i)
                    emit_drain(2, i, psA, an)
                    emit_transpose(i, an, 2, pst)
                    psA = emit_pv(3, i)
                    emit_drain(3, i, psA, an)
                    emit_transpose(i, an, 3, pst)
                    for ft, nt in sprinkle[len(sprinkle) // 2:]:
                        emit_qk(ft, nt, on_act=(ft % 2 == 0))
                    if i >= 1:
                        yt = y_pool.tile([P, C], BF16, tag="y", name="y")
                        emit_proj(i - 1, 0, yt)
                        emit_proj(i - 1, 1, yt)

                yt = y_pool.tile([P, C], BF16, tag="y", name="y")
                emit_proj(NT - 1, 0, yt)
                emit_proj(NT - 1, 1, yt)

    nc.finalize()
    return nc


def _band_mask_strip() -> np.ndarray:
    tk = np.arange(P)[:, None]
    tq = np.arange(P)[None, :]
    tri = (tq >= tk)            # dj=0 block: lower-incl triangle valid
    striu = (tq < tk)           # dj=2 block: strict upper triangle valid
    return np.concatenate([tri, striu], axis=1).astype(ml_dtypes.bfloat16)


def _pack_rows(a: np.ndarray) -> np.ndarray:
    """[n*P, F] -> [P, n*F] with chunk-major free dim (chunk k at cols k*F)."""
    n = a.shape[0] // P
    return np.ascontiguousarray(
        a.reshape(n, P, a.shape[1]).transpose(1, 0, 2).reshape(P, -1)
    )


def kernel(x, W_attn, b_attn, W_proj, b_proj):
    global LAST_RESULTS
    x = np.asarray(x, dtype=np.float32)
    W_attn = np.asarray(W_attn, dtype=np.float32)
    b_attn = np.asarray(b_attn, dtype=np.float32)
    W_proj = np.asarray(W_proj, dtype=np.float32)
    b_proj = np.asarray(b_proj, dtype=np.float32)

    qk_bias = bool(np.any(b_attn[:2 * C]))
    v_bias = bool(np.any(b_attn[2 * C:]))

    key = (qk_bias, v_bias)
    if key not in _BUILD_CACHE:
        _BUILD_CACHE[key] = _build_nc(qk_bias, v_bias)
    nc = _BUILD_CACHE[key]

    mstrip = _band_mask_strip()
    in_maps = []
    for c in range(8):
        b, g = c // 2, c % 2
        fsl = slice(FG * g, FG * (g + 1))
        # xt: [C, T] -> [P, tier(4) k(8) t(512)]
        xtp = np.ascontiguousarray(
            x[b].T.reshape(KC, P, 4, 512).transpose(1, 2, 0, 3).reshape(P, KC * T)
        )
        im = {
            "xt": xtp.astype(ml_dtypes.bfloat16),
            "wqkq": _pack_rows(W_attn[:, fsl]).astype(ml_dtypes.bfloat16),
            "wqkk": _pack_rows(
                W_attn[:, C + FG * g:C + FG * (g + 1)]
            ).astype(ml_dtypes.bfloat16),
            "wv": _pack_rows(
                W_attn[:, 2 * C + FG * g:2 * C + FG * (g + 1)]
            ).astype(ml_dtypes.bfloat16),
            "wp": _pack_rows(W_proj[fsl, :]).astype(ml_dtypes.bfloat16),
            "mstrip": mstrip,
            "ident": np.eye(P, dtype=ml_dtypes.bfloat16),
        }
        if qk_bias:
            bq = b_attn[fsl]
            bk = b_attn[C + FG * g:C + FG * (g + 1)]
            im["bqk"] = np.concatenate([bq, bk]).reshape(8, P).astype(np.float32)
        if v_bias:
            bv = b_attn[2 * C + FG * g:2 * C + FG * (g + 1)]
            bvt = np.zeros((HG, D + 1), dtype=np.float32)
            bvt[:, 1:] = bv.reshape(HG, D)
            im["bv"] = np.broadcast_to(
                bvt.reshape(1, HG * (D + 1)), (P, HG * (D + 1))
            ).astype(ml_dtypes.bfloat16)
        in_maps.append(im)

    res = run_bass_kernel_spmd(nc, in_maps, list(range(8)))
    LAST_RESULTS = res

    out = np.empty((B, T, C), dtype=np.float32)
    for b in range(B):
        out[b] = (res.results[2 * b]["y"].astype(np.float32)
                  + res.results[2 * b + 1]["y"].astype(np.float32) + b_proj)
    return out
